# revision 26
# baseline (speedup 1.0000x reference)
"""Trainium2 Bass kernel for nn_BaseTimeAttention (dense transformer block:
QKV projection + RoPE + softmax attention + output projection).

Problem (hardcoded):
  x:  [B=2, S=2048, H=2048] fp32,  Wq/Wk/Wv/Wo: [2048, 2048] fp32
  out = softmax((rope(xWq^T) rope(xWk^T)^T)/sqrt(128)) (xWv^T) Wo^T

Sharding (8 cores): tensor-parallel over heads x data-parallel over batch.
Core c handles batch b=c//4 and head group g=c%4 (4 of 16 heads = 512 of 2048
channels). Each core produces a full [2048, 2048] partial of the output
projection restricted to its 512 input channels; the host sums 4 partials per
batch (o_proj row-parallel reduction on host).

V1 changes vs baseline (532us):
- All inputs converted to bf16 on the host: same PE rate (1 col/cycle), half
  the DMA bytes and half the SBUF footprint. rel-err budget is 2e-2; measured
  fp32r error was 7e-4, bf16 error ~1e-2 expected worst case.
- Q^T/K^T/V stay SBUF-resident between phases (bf16: 6MB total) - no DRAM
  round trip, no phase-1->2 reload stall.
- Softmax denominator moved off the PE: VectorE pairwise-adds the exp tiles
  (bf16, 2x rate) and a single ones-matmul per (head, query-block) does the
  final partition reduction (was 16 matmuls -> now 1: saves ~52us of PE).
- Output projection interleaved into phase 2 per query block: o-proj matmuls
  for block n run while ScalarE exps block n+1, keeping the PE busy.
- RoPE consumes PSUM directly on VectorE (no ScalarE pre-copy for the cos
  path); the rotate-half partner still comes from an SBUF->SBUF DMA swap with
  the sign folded into the host-built sin table.
"""

import numpy as np
import ml_dtypes

import concourse.mybir as mybir
import concourse.tile as tile
from concourse import bacc
from concourse.bass_utils import run_bass_kernel_spmd

F32 = mybir.dt.float32
BF16 = mybir.dt.bfloat16
AF = mybir.ActivationFunctionType
NPBF16 = ml_dtypes.bfloat16

B = 2
S = 2048
HIDDEN = 2048
HEADS = 16
DH = 128
THETA = 10000.0
N_CORES = 8
GROUPS = 4
HPC = HEADS // GROUPS  # heads per core
JPC = HPC * DH  # projection cols per core
SCALE = 1.0 / np.sqrt(DH)

SB = 512
NSB = S // SB
KT = HIDDEN // 128  # 16 contraction tiles
NKT = S // 128  # 16 s_k tiles

# half-tensor chunks: the HWDGE rings serialize transfers (~3us fixed + line
# rate each), so the startup working set (x0 + wq, 4MB) is split k-wise across
# BOTH rings - all four halves land in parallel by ~14us, which the HAM
# warm-up matmuls cover.
CHS = (8, 8)
CHO = (0, 8)


def build():
    nc = bacc.Bacc("TRN2", target_bir_lowering=False, debug=False)

    # partition-major inputs (see _make_in_maps), all bf16
    x_d = nc.dram_tensor("xPM", [NSB, 128, KT, SB], BF16, kind="ExternalInput")
    wq_d = nc.dram_tensor("wqPM", [128, KT, JPC], BF16, kind="ExternalInput")
    wk_d = nc.dram_tensor("wkPM", [128, KT, JPC], BF16, kind="ExternalInput")
    wv_d = nc.dram_tensor("wvPM", [128, KT, JPC], BF16, kind="ExternalInput")
    wo_d = nc.dram_tensor("woPM", [128, HPC, HIDDEN], BF16, kind="ExternalInput")
    cos_d = nc.dram_tensor("cos", [DH, S], BF16, kind="ExternalInput")
    sin_d = nc.dram_tensor("sinS", [DH, S], BF16, kind="ExternalInput")
    ones_d = nc.dram_tensor("ones", [128, 128], BF16, kind="ExternalInput")
    out_d = nc.dram_tensor("out", [S, HIDDEN], BF16, kind="ExternalOutput")

    out = out_d.ap()

    with tile.TileContext(nc) as tc:
        with tc.tile_pool(name="persist", bufs=1) as persist:
            q_sb = persist.tile([128, HPC, S], BF16, tag="q")
            k_sb = persist.tile([128, HPC, S], BF16, tag="k")
            v_sb = persist.tile([128, HPC, NKT, DH], BF16, tag="v")
            wo = persist.tile([128, HPC, HIDDEN], BF16, tag="wo")
            cos_sb = persist.tile([128, S], BF16, tag="cos")
            sin_sb = persist.tile([128, S], BF16, tag="sin")
            ones_sb = persist.tile([128, 128], BF16, tag="ones")
            yt = persist.tile([128, HPC, S], BF16, tag="yt")

            # ---------------- Phase 1: projections + RoPE ------------------
            # s-block outer, projection inner: x is streamed from HBM once.
            # sync ring: wq, wk chunks + rotate-half swaps (later: out stores)
            # scalar ring: cos/sin/ones, x blocks, wv, wo
            def wslice(chunks, k, cols):
                for c in range(len(CHS)):
                    if k < CHO[c] + CHS[c]:
                        return chunks[c][:, k - CHO[c], cols]
                raise AssertionError

            with (
                tc.tile_pool(name="p1w", bufs=1) as p1w,
                tc.tile_pool(name="p1x", bufs=2) as p1x,
                tc.tile_pool(name="p1s", bufs=4) as p1s,
                tc.tile_pool(name="p1ps", bufs=2, space="PSUM") as p1ps,
            ):
                # HAM warm-up: the PE clock-gate defaults to 1.2 GHz and takes
                # ~3.4us of sustained activity to release. The PE idles ~8us
                # waiting for the first DMAs anyway, so burn that window on
                # dummy matmuls to enter phase 1 at the full 2.4 GHz. Tiles
                # come from the existing pools: a dedicated pool scope would
                # close with a barrier that delays the phase-1 DMA issues.
                wtile = p1s.tile([128, SB], BF16, tag="praw")
                nc.gpsimd.memset(wtile[:], 0)
                wps = p1ps.tile([128, SB], F32, tag="ps")
                for _ in range(20):
                    nc.tensor.matmul(
                        wps[:], wtile[:, 0:128], wtile[:], start=True, stop=True
                    )
                def load_xs(s, engs):
                    xsc = []
                    for c in range(len(CHS)):
                        xt = p1x.tile([128, CHS[c], SB], BF16, tag=f"xs{c}")
                        engs[c].dma_start(
                            out=xt[:],
                            in_=x_d.ap()[s, :, CHO[c] : CHO[c] + CHS[c], :],
                        )
                        xsc.append(xt)
                    return xsc

                wchunks = {}

                def load_w(name, w_d, engs):
                    for c in range(len(CHS)):
                        w = p1w.tile([128, CHS[c], JPC], BF16, tag=f"w{name}{c}")
                        engs[c].dma_start(
                            out=w[:],
                            in_=w_d.ap()[:, CHO[c] : CHO[c] + CHS[c], :],
                        )
                        wchunks.setdefault(name, []).append(w)

                # startup: x0/wq halves split across the two HWDGE rings so
                # all four 1MB transfers run concurrently.
                #   sync ring:   x0-lo, wq-hi, wk-lo, wk-hi, (swaps...)
                #   scalar ring: wq-lo, x0-hi, cos, sin, wv, wo, ones, x1..3
                xs_next = load_xs(0, (nc.sync, nc.scalar))
                load_w("q", wq_d, (nc.scalar, nc.sync))
                load_w("k", wk_d, (nc.sync, nc.scalar))
                nc.scalar.dma_start(out=cos_sb[:], in_=cos_d.ap())
                nc.scalar.dma_start(out=sin_sb[:], in_=sin_d.ap())
                load_w("v", wv_d, (nc.scalar, nc.scalar))
                nc.scalar.dma_start(out=wo[:], in_=wo_d.ap())
                nc.scalar.dma_start(out=ones_sb[:], in_=ones_d.ap())

                for s in range(NSB):
                    sblk = slice(s * SB, (s + 1) * SB)
                    xsc = xs_next
                    if s + 1 < NSB:
                        xs_next = load_xs(s + 1, (nc.scalar, nc.scalar))
                    for name in ("q", "k", "v"):
                        for j in range(HPC):
                            jblk = slice(j * 128, (j + 1) * 128)
                            if name != "v":
                                ps = p1ps.tile([128, SB], F32, tag="ps")
                            else:
                                ps = p1ps.tile([128, HPC, DH], F32, tag="psv")
                            for k in range(KT):
                                if name != "v":  # Q/K: [j, s] transposed
                                    lhsT = wslice(wchunks[name], k, jblk)
                                    rhs = wslice(xsc, k, slice(0, SB))
                                else:  # V: natural [s, j]
                                    lhsT = wslice(xsc, k, jblk)
                                    rhs = wslice(wchunks[name], k, slice(0, JPC))
                                nc.tensor.matmul(
                                    ps[:],
                                    lhsT,
                                    rhs,
                                    start=(k == 0),
                                    stop=(k == KT - 1),
                                )
                            if name == "v":
                                # strided copy: [128 s, (h d)] -> v_sb[:, h, idx, d]
                                # (VectorE: the Scalar queue head-of-line
                                # blocks on DMA ring credits in phase 1)
                                nc.vector.tensor_copy(
                                    v_sb[:, :, s * HPC + j, :], ps[:]
                                )
                            else:
                                dst = q_sb if name == "q" else k_sb
                                praw = p1s.tile([128, SB], BF16, tag="praw")
                                qc = p1s.tile([128, SB], BF16, tag="qc")
                                tmp = p1s.tile([128, SB], BF16, tag="tmp")
                                tmp2 = p1s.tile([128, SB], BF16, tag="tmp2")
                                nc.vector.tensor_copy(praw[:], ps[:])
                                nc.vector.tensor_mul(
                                    qc[:], ps[:], cos_sb[:, sblk]
                                )
                                nc.sync.dma_start(
                                    out=tmp[0:64, :], in_=praw[64:128, :]
                                )
                                nc.sync.dma_start(
                                    out=tmp[64:128, :], in_=praw[0:64, :]
                                )
                                nc.vector.tensor_mul(
                                    tmp2[:], tmp[:], sin_sb[:, sblk]
                                )
                                nc.vector.tensor_add(
                                    dst[:, j, sblk], qc[:], tmp2[:]
                                )

            # ---------- Phase 2+3: attention + interleaved o-proj ----------
            with (
                tc.tile_pool(name="p2e", bufs=18) as p2e,
                tc.tile_pool(name="p2d", bufs=2) as p2d,
                tc.tile_pool(name="p2r", bufs=2) as p2r,
                tc.tile_pool(name="p3s", bufs=4) as p3s,
                tc.tile_pool(name="p2sc", bufs=2, space="PSUM") as p2sc,
                tc.tile_pool(name="pacc", bufs=3, space="PSUM") as pacc,
                tc.tile_pool(name="pden", bufs=1, space="PSUM") as pden,
            ):
                NP = NKT // 2  # 8 score pairs
                PIPE = 2

                def oproj_thunks(n):
                    thunks = []
                    for m4 in range(4):
                        mblk = slice(n * SB + m4 * 128, n * SB + (m4 + 1) * 128)
                        for nn in range(4):
                            nnblk = slice(nn * SB, (nn + 1) * SB)

                            def w(mblk=mblk, nnblk=nnblk):
                                pso = pacc.tile([128, SB], F32, tag="acc")
                                for kj in range(HPC):
                                    nc.tensor.matmul(
                                        pso[:],
                                        yt[:, kj, mblk],
                                        wo[:, kj, nnblk],
                                        start=(kj == 0),
                                        stop=(kj == HPC - 1),
                                    )
                                oc = p3s.tile([128, SB], BF16, tag="oc")
                                nc.vector.tensor_copy(oc[:], pso[:])
                                nc.sync.dma_start(out=out[mblk, nnblk], in_=oc[:])

                            thunks.append(w)
                    return thunks

                def unit(h, n, oproj_work):
                    nblk = slice(n * SB, (n + 1) * SB)
                    num = pacc.tile([128, SB], F32, tag="acc")
                    den = pden.tile([128, SB], F32, tag="den")
                    es = [None] * NP
                    acc = None
                    for p in range(NP + PIPE):
                        if p < NP:
                            sc2 = p2sc.tile([128, 2, SB], F32, tag="sc")
                            e2 = p2e.tile([128, 2, SB], BF16, tag="e")
                            for half in range(2):
                                i = 2 * p + half
                                nc.tensor.matmul(
                                    sc2[:, half, :],
                                    k_sb[:, h, i * 128 : (i + 1) * 128],
                                    q_sb[:, h, nblk],
                                    start=True,
                                    stop=True,
                                )
                            nc.scalar.activation(
                                e2[:], sc2[:], AF.Exp, scale=float(SCALE)
                            )
                            es[p] = e2
                            # incremental denominator accumulation (VectorE,
                            # bf16 2x) so nothing serializes at unit end
                            if p == 1:
                                acc = p2d.tile([128, 2, SB], BF16, tag="dacc")
                                nc.vector.tensor_add(
                                    acc[:], es[0][:], es[1][:]
                                )
                            elif p > 1:
                                nxt = p2d.tile([128, 2, SB], BF16, tag="dacc")
                                nc.vector.tensor_add(nxt[:], acc[:], e2[:])
                                acc = nxt
                        if p == 1 and oproj_work:
                            # o-proj matmuls of the previous query block fill
                            # the PE while ScalarE exps this unit's scores
                            for wrk in oproj_work:
                                wrk()
                            oproj_work.clear()
                        if p >= PIPE:
                            pp = p - PIPE
                            for half in range(2):
                                i = 2 * pp + half
                                nc.tensor.matmul(
                                    num[:],
                                    v_sb[:, h, i, :],
                                    es[pp][:, half, :],
                                    start=(i == 0),
                                    stop=(i == NKT - 1),
                                )
                    accq = p2d.tile([128, SB], BF16, tag="daccq")
                    nc.vector.tensor_add(accq[:], acc[:, 0, :], acc[:, 1, :])
                    nc.tensor.matmul(
                        den[:], ones_sb[:], accq[:], start=True, stop=True
                    )
                    r = p2r.tile([128, SB], F32, tag="r")
                    nc.vector.reciprocal_approx_fast(out=r[:], in_=den[:])
                    if n == NSB - 1 and h == HPC - 1:
                        # last unit: split so the final o-proj block's first
                        # m-tile starts as soon as its 128 columns are ready
                        for m4 in range(4):
                            cb = slice(m4 * 128, (m4 + 1) * 128)
                            sub = slice(n * SB + m4 * 128, n * SB + (m4 + 1) * 128)
                            nc.vector.tensor_mul(
                                yt[:, h, sub], num[:, cb], r[:, cb]
                            )
                    else:
                        nc.vector.tensor_mul(yt[:, h, nblk], num[:], r[:])

                pending = []
                for n in range(NSB):
                    for h in range(HPC):
                        unit(h, n, pending if h == 0 else None)
                    pending = oproj_thunks(n)
                for wrk in pending:
                    wrk()

    nc.compile()
    return nc


_NC = None


def _get_nc():
    global _NC
    if _NC is None:
        _NC = build()
    return _NC


def _rope_tables():
    inv_freq = 1.0 / (THETA ** (np.arange(0, DH, 2, dtype=np.float32) / DH))
    freqs = np.arange(S, dtype=np.float32)[:, None] * inv_freq[None, :]  # [S, 64]
    cos_h = np.cos(freqs).T.astype(np.float32)  # [64, S]
    sin_h = np.sin(freqs).T.astype(np.float32)
    cos = np.concatenate([cos_h, cos_h], axis=0)  # [128, S]
    sin_s = np.concatenate([-sin_h, sin_h], axis=0)  # [128, S]
    return np.ascontiguousarray(cos), np.ascontiguousarray(sin_s)


def _pm_weight(wT):  # [2048, 512] (k, j) -> [128, 16, 512] partition-major
    return np.ascontiguousarray(
        wT.reshape(KT, 128, JPC).transpose(1, 0, 2).astype(NPBF16)
    )


def _make_in_maps(inputs):
    x = np.asarray(inputs["x"], dtype=np.float32)
    Wq = np.asarray(inputs["Wq"], dtype=np.float32)
    Wk = np.asarray(inputs["Wk"], dtype=np.float32)
    Wv = np.asarray(inputs["Wv"], dtype=np.float32)
    Wo = np.asarray(inputs["Wo"], dtype=np.float32)

    cos, sin_s = _rope_tables()
    ones = np.ones((128, 128), dtype=np.float32)

    in_maps = []
    for c in range(N_CORES):
        b = c // GROUPS
        g = c % GROUPS
        rows = slice(g * JPC, (g + 1) * JPC)
        xT = x[b].T  # [hidden(k), s]
        # [k, s] -> [s_blk, p, kt, s_in_blk]
        xpm = np.ascontiguousarray(
            xT.reshape(KT, 128, NSB, SB).transpose(2, 1, 0, 3).astype(NPBF16)
        )
        # Wo[:, rows].T -> [512(j), 2048] -> [p, kj, 2048]
        woT = Wo[:, rows].T
        wopm = np.ascontiguousarray(
            woT.reshape(HPC, 128, HIDDEN).transpose(1, 0, 2).astype(NPBF16)
        )
        in_maps.append(
            {
                "xPM": xpm,
                "wqPM": _pm_weight(Wq[rows].T),
                "wkPM": _pm_weight(Wk[rows].T),
                "wvPM": _pm_weight(Wv[rows].T),
                "woPM": wopm,
                "cos": cos.astype(NPBF16),
                "sinS": sin_s.astype(NPBF16),
                "ones": ones.astype(NPBF16),
            }
        )
    return in_maps


def kernel(x, Wq, Wk, Wv, Wo):
    nc = _get_nc()
    in_maps = _make_in_maps({"x": x, "Wq": Wq, "Wk": Wk, "Wv": Wv, "Wo": Wo})
    res = run_bass_kernel_spmd(nc, in_maps, list(range(N_CORES)))

    out = np.zeros((B, S, HIDDEN), dtype=np.float32)
    for c in range(N_CORES):
        out[c // GROUPS] += np.asarray(res.results[c]["out"], dtype=np.float32)
    return out


# revision 28
# speedup vs baseline: 1.2176x; 1.2176x over previous
"""Trainium2 Bass kernel for nn_BaseTimeAttention (dense transformer block:
QKV projection + RoPE + softmax attention + output projection).

Problem (hardcoded):
  x:  [B=2, S=2048, H=2048] fp32,  Wq/Wk/Wv/Wo: [2048, 2048] fp32
  out = softmax((rope(xWq^T) rope(xWk^T)^T)/sqrt(128)) (xWv^T) Wo^T

Sharding (8 cores): tensor-parallel over heads x data-parallel over batch.
Core c handles batch b=c//4 and head group g=c%4 (4 of 16 heads = 512 of 2048
channels). Each core produces a full [2048, 2048] partial of the output
projection restricted to its 512 input channels; the host sums 4 partials per
batch (o_proj row-parallel reduction on host).

V1 changes vs baseline (532us):
- All inputs converted to bf16 on the host: same PE rate (1 col/cycle), half
  the DMA bytes and half the SBUF footprint. rel-err budget is 2e-2; measured
  fp32r error was 7e-4, bf16 error ~1e-2 expected worst case.
- Q^T/K^T/V stay SBUF-resident between phases (bf16: 6MB total) - no DRAM
  round trip, no phase-1->2 reload stall.
- Softmax denominator moved off the PE: VectorE pairwise-adds the exp tiles
  (bf16, 2x rate) and a single ones-matmul per (head, query-block) does the
  final partition reduction (was 16 matmuls -> now 1: saves ~52us of PE).
- Output projection interleaved into phase 2 per query block: o-proj matmuls
  for block n run while ScalarE exps block n+1, keeping the PE busy.
- RoPE consumes PSUM directly on VectorE (no ScalarE pre-copy for the cos
  path); the rotate-half partner still comes from an SBUF->SBUF DMA swap with
  the sign folded into the host-built sin table.
"""

import numpy as np
import ml_dtypes

import concourse.mybir as mybir
import concourse.tile as tile
from concourse import bacc
from concourse.bass_utils import run_bass_kernel_spmd

F32 = mybir.dt.float32
BF16 = mybir.dt.bfloat16
AF = mybir.ActivationFunctionType
NPBF16 = ml_dtypes.bfloat16

B = 2
S = 2048
HIDDEN = 2048
HEADS = 16
DH = 128
THETA = 10000.0
N_CORES = 8
GROUPS = 4
HPC = HEADS // GROUPS  # heads per core
JPC = HPC * DH  # projection cols per core
SCALE = 1.0 / np.sqrt(DH)

SB = 512
NSB = S // SB
KT = HIDDEN // 128  # 16 contraction tiles
NKT = S // 128  # 16 s_k tiles

# half-tensor chunks: the HWDGE rings serialize transfers (~3us fixed + line
# rate each), so the startup working set (x0 + wq, 4MB) is split k-wise across
# BOTH rings - all four halves land in parallel by ~14us, which the HAM
# warm-up matmuls cover.
CHS = (8, 8)
CHO = (0, 8)


def build():
    nc = bacc.Bacc("TRN2", target_bir_lowering=False, debug=False)

    # partition-major inputs (see _make_in_maps), all bf16
    x_d = nc.dram_tensor("xPM", [NSB, 128, KT, SB], BF16, kind="ExternalInput")
    wq_d = nc.dram_tensor("wqPM", [128, KT, JPC], BF16, kind="ExternalInput")
    wk_d = nc.dram_tensor("wkPM", [128, KT, JPC], BF16, kind="ExternalInput")
    wv_d = nc.dram_tensor("wvPM", [128, KT, JPC], BF16, kind="ExternalInput")
    wo_d = nc.dram_tensor("woPM", [128, HPC, HIDDEN], BF16, kind="ExternalInput")
    cos_d = nc.dram_tensor("cos", [DH, S], BF16, kind="ExternalInput")
    sin_d = nc.dram_tensor("sinS", [DH, S], BF16, kind="ExternalInput")
    ones_d = nc.dram_tensor("ones", [128, 128], BF16, kind="ExternalInput")
    out_d = nc.dram_tensor("out", [S, HIDDEN], BF16, kind="ExternalOutput")

    out = out_d.ap()

    with tile.TileContext(nc) as tc:
        with tc.tile_pool(name="persist", bufs=1) as persist:
            q_sb = persist.tile([128, HPC, S], BF16, tag="q")
            k_sb = persist.tile([128, HPC, S], BF16, tag="k")
            v_sb = persist.tile([128, HPC, NKT, DH], BF16, tag="v")
            wo = persist.tile([128, HPC, HIDDEN], BF16, tag="wo")
            cos_sb = persist.tile([128, S], BF16, tag="cos")
            sin_sb = persist.tile([128, S], BF16, tag="sin")
            ones_sb = persist.tile([128, 128], BF16, tag="ones")
            yt = persist.tile([128, HPC, S], BF16, tag="yt")

            # ---------------- Phase 1: projections + RoPE ------------------
            # s-block outer, projection inner: x is streamed from HBM once.
            # sync ring: wq, wk chunks + rotate-half swaps (later: out stores)
            # scalar ring: cos/sin/ones, x blocks, wv, wo
            def wslice(chunks, k, cols):
                for c in range(len(CHS)):
                    if k < CHO[c] + CHS[c]:
                        return chunks[c][:, k - CHO[c], cols]
                raise AssertionError

            with (
                tc.tile_pool(name="p1w", bufs=1) as p1w,
                tc.tile_pool(name="p1x", bufs=2) as p1x,
                tc.tile_pool(name="p1s", bufs=4) as p1s,
                tc.tile_pool(name="p1ps", bufs=2, space="PSUM") as p1ps,
            ):
                # HAM warm-up: the PE clock-gate defaults to 1.2 GHz and takes
                # ~3.4us of sustained activity to release. The PE idles ~8us
                # waiting for the first DMAs anyway, so burn that window on
                # dummy matmuls to enter phase 1 at the full 2.4 GHz. Tiles
                # come from the existing pools: a dedicated pool scope would
                # close with a barrier that delays the phase-1 DMA issues.
                wtile = p1s.tile([128, SB], BF16, tag="praw")
                nc.gpsimd.memset(wtile[:], 0)
                wps = p1ps.tile([128, SB], F32, tag="ps")
                for _ in range(20):
                    nc.tensor.matmul(
                        wps[:], wtile[:, 0:128], wtile[:], start=True, stop=True
                    )
                def load_xs(s, engs):
                    xsc = []
                    for c in range(len(CHS)):
                        xt = p1x.tile([128, CHS[c], SB], BF16, tag=f"xs{c}")
                        engs[c].dma_start(
                            out=xt[:],
                            in_=x_d.ap()[s, :, CHO[c] : CHO[c] + CHS[c], :],
                        )
                        xsc.append(xt)
                    return xsc

                wchunks = {}

                def load_w(name, w_d, engs):
                    for c in range(len(CHS)):
                        w = p1w.tile([128, CHS[c], JPC], BF16, tag=f"w{name}{c}")
                        engs[c].dma_start(
                            out=w[:],
                            in_=w_d.ap()[:, CHO[c] : CHO[c] + CHS[c], :],
                        )
                        wchunks.setdefault(name, []).append(w)

                # startup: x0/wq halves split across the two HWDGE rings so
                # all four 1MB transfers run concurrently.
                #   sync ring:   x0-lo, wq-hi, wk-lo, wk-hi, (swaps...)
                #   scalar ring: wq-lo, x0-hi, cos, sin, wv, wo, ones, x1..3
                xs_next = load_xs(0, (nc.sync, nc.scalar))
                load_w("q", wq_d, (nc.scalar, nc.sync))
                load_w("k", wk_d, (nc.sync, nc.scalar))
                nc.scalar.dma_start(out=cos_sb[:], in_=cos_d.ap())
                nc.scalar.dma_start(out=sin_sb[:], in_=sin_d.ap())
                load_w("v", wv_d, (nc.scalar, nc.scalar))
                nc.scalar.dma_start(out=wo[:], in_=wo_d.ap())
                nc.scalar.dma_start(out=ones_sb[:], in_=ones_d.ap())

                for s in range(NSB):
                    sblk = slice(s * SB, (s + 1) * SB)
                    xsc = xs_next
                    if s + 1 < NSB:
                        xs_next = load_xs(s + 1, (nc.scalar, nc.scalar))
                    for name in ("q", "k", "v"):
                        for j in range(HPC):
                            jblk = slice(j * 128, (j + 1) * 128)
                            if name != "v":
                                ps = p1ps.tile([128, SB], F32, tag="ps")
                            else:
                                ps = p1ps.tile([128, HPC, DH], F32, tag="psv")
                            for k in range(KT):
                                if name != "v":  # Q/K: [j, s] transposed
                                    lhsT = wslice(wchunks[name], k, jblk)
                                    rhs = wslice(xsc, k, slice(0, SB))
                                else:  # V: natural [s, j]
                                    lhsT = wslice(xsc, k, jblk)
                                    rhs = wslice(wchunks[name], k, slice(0, JPC))
                                nc.tensor.matmul(
                                    ps[:],
                                    lhsT,
                                    rhs,
                                    start=(k == 0),
                                    stop=(k == KT - 1),
                                )
                            if name == "v":
                                # strided copy: [128 s, (h d)] -> v_sb[:, h, idx, d]
                                # (VectorE: the Scalar queue head-of-line
                                # blocks on DMA ring credits in phase 1)
                                nc.vector.tensor_copy(
                                    v_sb[:, :, s * HPC + j, :], ps[:]
                                )
                            else:
                                dst = q_sb if name == "q" else k_sb
                                praw = p1s.tile([128, SB], BF16, tag="praw")
                                qc = p1s.tile([128, SB], BF16, tag="qc")
                                tmp = p1s.tile([128, SB], BF16, tag="tmp")
                                tmp2 = p1s.tile([128, SB], BF16, tag="tmp2")
                                nc.vector.tensor_copy(praw[:], ps[:])
                                nc.vector.tensor_mul(
                                    qc[:], ps[:], cos_sb[:, sblk]
                                )
                                nc.sync.dma_start(
                                    out=tmp[0:64, :], in_=praw[64:128, :]
                                )
                                nc.sync.dma_start(
                                    out=tmp[64:128, :], in_=praw[0:64, :]
                                )
                                nc.vector.tensor_mul(
                                    tmp2[:], tmp[:], sin_sb[:, sblk]
                                )
                                nc.vector.tensor_add(
                                    dst[:, j, sblk], qc[:], tmp2[:]
                                )

            # ---------- Phase 2+3: attention + interleaved o-proj ----------
            with (
                tc.tile_pool(name="p2e", bufs=18) as p2e,
                tc.tile_pool(name="p2d", bufs=2) as p2d,
                tc.tile_pool(name="p2r", bufs=2) as p2r,
                tc.tile_pool(name="p3s", bufs=4) as p3s,
                tc.tile_pool(name="p2sc", bufs=2, space="PSUM") as p2sc,
                tc.tile_pool(name="pacc", bufs=3, space="PSUM") as pacc,
                tc.tile_pool(name="pden", bufs=1, space="PSUM") as pden,
            ):
                NP = NKT // 2  # 8 score pairs
                PIPE = 2

                def oproj_thunks(n):
                    thunks = []
                    for m4 in range(4):
                        mblk = slice(n * SB + m4 * 128, n * SB + (m4 + 1) * 128)
                        for nn in range(4):
                            nnblk = slice(nn * SB, (nn + 1) * SB)

                            def w(mblk=mblk, nnblk=nnblk):
                                pso = pacc.tile([128, SB], F32, tag="acc")
                                for kj in range(HPC):
                                    nc.tensor.matmul(
                                        pso[:],
                                        yt[:, kj, mblk],
                                        wo[:, kj, nnblk],
                                        start=(kj == 0),
                                        stop=(kj == HPC - 1),
                                    )
                                oc = p3s.tile([128, SB], BF16, tag="oc")
                                nc.vector.tensor_copy(oc[:], pso[:])
                                nc.sync.dma_start(out=out[mblk, nnblk], in_=oc[:])

                            thunks.append(w)
                    return thunks

                def unit(h, n, oproj_work):
                    nblk = slice(n * SB, (n + 1) * SB)
                    num = pacc.tile([128, SB], F32, tag="acc")
                    den = pden.tile([128, SB], F32, tag="den")
                    es = [None] * NP
                    acc = None
                    for p in range(NP + PIPE):
                        if p < NP:
                            sc2 = p2sc.tile([128, 2, SB], F32, tag="sc")
                            e2 = p2e.tile([128, 2, SB], BF16, tag="e")
                            for half in range(2):
                                i = 2 * p + half
                                nc.tensor.matmul(
                                    sc2[:, half, :],
                                    k_sb[:, h, i * 128 : (i + 1) * 128],
                                    q_sb[:, h, nblk],
                                    start=True,
                                    stop=True,
                                )
                            nc.scalar.activation(
                                e2[:], sc2[:], AF.Exp, scale=float(SCALE)
                            )
                            es[p] = e2
                            # incremental denominator accumulation (VectorE,
                            # bf16 2x) so nothing serializes at unit end
                            if p == 1:
                                acc = p2d.tile([128, 2, SB], BF16, tag="dacc")
                                nc.vector.tensor_add(
                                    acc[:], es[0][:], es[1][:]
                                )
                            elif p > 1:
                                nxt = p2d.tile([128, 2, SB], BF16, tag="dacc")
                                nc.vector.tensor_add(nxt[:], acc[:], e2[:])
                                acc = nxt
                        if oproj_work and p in (1, 3, 5, 7):
                            # one o-proj tile of the previous query block
                            # after every other score pair: dependency-free
                            # PE filler so the exp pacing (ScalarE ~1.0us vs
                            # PE ~0.86us per pair) never idles the PE (idle
                            # dips also re-throttle the HAM clock gate)
                            oproj_work.pop(0)()
                        if p >= PIPE:
                            pp = p - PIPE
                            for half in range(2):
                                i = 2 * pp + half
                                nc.tensor.matmul(
                                    num[:],
                                    v_sb[:, h, i, :],
                                    es[pp][:, half, :],
                                    start=(i == 0),
                                    stop=(i == NKT - 1),
                                )
                    accq = p2d.tile([128, SB], BF16, tag="daccq")
                    nc.vector.tensor_add(accq[:], acc[:, 0, :], acc[:, 1, :])
                    nc.tensor.matmul(
                        den[:], ones_sb[:], accq[:], start=True, stop=True
                    )
                    r = p2r.tile([128, SB], F32, tag="r")
                    nc.vector.reciprocal_approx_fast(out=r[:], in_=den[:])
                    if n == NSB - 1 and h == HPC - 1:
                        # last unit: split so the final o-proj block's first
                        # m-tile starts as soon as its 128 columns are ready
                        for m4 in range(4):
                            cb = slice(m4 * 128, (m4 + 1) * 128)
                            sub = slice(n * SB + m4 * 128, n * SB + (m4 + 1) * 128)
                            nc.vector.tensor_mul(
                                yt[:, h, sub], num[:, cb], r[:, cb]
                            )
                    else:
                        nc.vector.tensor_mul(yt[:, h, nblk], num[:], r[:])

                pending = []
                for n in range(NSB):
                    for h in range(HPC):
                        unit(h, n, pending)
                    pending = oproj_thunks(n)
                for wrk in pending:
                    wrk()

    nc.compile()
    return nc


_NC = None


def _get_nc():
    global _NC
    if _NC is None:
        _NC = build()
    return _NC


def _rope_tables():
    inv_freq = 1.0 / (THETA ** (np.arange(0, DH, 2, dtype=np.float32) / DH))
    freqs = np.arange(S, dtype=np.float32)[:, None] * inv_freq[None, :]  # [S, 64]
    cos_h = np.cos(freqs).T.astype(np.float32)  # [64, S]
    sin_h = np.sin(freqs).T.astype(np.float32)
    cos = np.concatenate([cos_h, cos_h], axis=0)  # [128, S]
    sin_s = np.concatenate([-sin_h, sin_h], axis=0)  # [128, S]
    return np.ascontiguousarray(cos), np.ascontiguousarray(sin_s)


def _pm_weight(wT):  # [2048, 512] (k, j) -> [128, 16, 512] partition-major
    return np.ascontiguousarray(
        wT.reshape(KT, 128, JPC).transpose(1, 0, 2).astype(NPBF16)
    )


def _make_in_maps(inputs):
    x = np.asarray(inputs["x"], dtype=np.float32)
    Wq = np.asarray(inputs["Wq"], dtype=np.float32)
    Wk = np.asarray(inputs["Wk"], dtype=np.float32)
    Wv = np.asarray(inputs["Wv"], dtype=np.float32)
    Wo = np.asarray(inputs["Wo"], dtype=np.float32)

    cos, sin_s = _rope_tables()
    ones = np.ones((128, 128), dtype=np.float32)

    in_maps = []
    for c in range(N_CORES):
        b = c // GROUPS
        g = c % GROUPS
        rows = slice(g * JPC, (g + 1) * JPC)
        xT = x[b].T  # [hidden(k), s]
        # [k, s] -> [s_blk, p, kt, s_in_blk]
        xpm = np.ascontiguousarray(
            xT.reshape(KT, 128, NSB, SB).transpose(2, 1, 0, 3).astype(NPBF16)
        )
        # Wo[:, rows].T -> [512(j), 2048] -> [p, kj, 2048]
        woT = Wo[:, rows].T
        wopm = np.ascontiguousarray(
            woT.reshape(HPC, 128, HIDDEN).transpose(1, 0, 2).astype(NPBF16)
        )
        in_maps.append(
            {
                "xPM": xpm,
                "wqPM": _pm_weight(Wq[rows].T),
                "wkPM": _pm_weight(Wk[rows].T),
                "wvPM": _pm_weight(Wv[rows].T),
                "woPM": wopm,
                "cos": cos.astype(NPBF16),
                "sinS": sin_s.astype(NPBF16),
                "ones": ones.astype(NPBF16),
            }
        )
    return in_maps


def kernel(x, Wq, Wk, Wv, Wo):
    nc = _get_nc()
    in_maps = _make_in_maps({"x": x, "Wq": Wq, "Wk": Wk, "Wv": Wv, "Wo": Wo})
    res = run_bass_kernel_spmd(nc, in_maps, list(range(N_CORES)))

    out = np.zeros((B, S, HIDDEN), dtype=np.float32)
    for c in range(N_CORES):
        out[c // GROUPS] += np.asarray(res.results[c]["out"], dtype=np.float32)
    return out
